# revision 6
# baseline (speedup 1.0000x reference)
"""Multi-head attention kernel for 8 Trainium2 NeuronCores.

Problem: B=4, S=2048, D=H=1024, NH=16 heads (head_dim 64), causal MHA with
input projections (W_q/W_k/W_v), softmax, and output projection (W_o).

Sharding: 8 cores = 4 batches x 2 head-groups (tensor parallel over heads).
Each core computes, for one batch b and one group g of 8 heads:
  QT/KT = (x @ W{q,k}[g].T + b).T  stored [feature, seq]   (column-parallel)
  V     = x @ Wv[g].T              stored [seq, feature]
  per head: P.T = exp((K_h.T Q_h)/8 + causal_mask)  [k, q]
            O.T = V_aug_h.T @ P.T  (V augmented with a ones column -> row 64
                                    of the product is the softmax denominator)
  partial.T = (W_o[:, g] slice) contracted with normalized O.T (row-parallel)
The host sums the two group partials per batch, transposes, and adds
(W_o @ b_v + b_o)  (exact folding of the v/out biases).

All matmuls run in float32r (fp32 rounded to 11 explicit mantissa bits,
1 cycle/row on the PE when the moving dim >= 256 -> 4x faster than fp32,
~2.4e-4 relative rounding). Inputs are pre-rounded on the host so plain
HWDGE DMAs (no casting, no SWDGE descriptor-generation serialization) load
them.
"""

import sys

if "/opt/trn_rl_repo" not in sys.path:
    sys.path.insert(0, "/opt/trn_rl_repo")

import numpy as np

import concourse.mybir as mybir
import concourse.tile as tile
from concourse import bacc
from concourse.bass_utils import run_bass_kernel_spmd

F32 = mybir.dt.float32
F32R = mybir.dt.float32r
EXP = mybir.ActivationFunctionType.Exp

# Problem dims (full) and per-core dims
B, S, D, H, NH, HD = 4, 2048, 1024, 1024, 16, 64
HL = H // 2          # per-core feature width (8 heads x 64)
NHL = HL // HD       # 8 local heads
D_TILES = D // 128   # 8
J_TILES = HL // 128  # 4
S_TILES = S // 128   # 16
QC = S // 1024       # 2 big q-chunks (2 PSUM banks wide)
NEG = -1.0e30

_nc_cache = {}


def _build_nc(upto="all"):
    key = f"nc_{upto}"
    if key in _nc_cache:
        return _nc_cache[key]

    nc = bacc.Bacc("TRN2", target_bir_lowering=False, debug=False)

    # fp32r inputs are pre-rounded fp32 bits from the host
    xq_t = nc.dram_tensor("xq_t", [D, S], F32R, kind="ExternalInput")
    xk_t = nc.dram_tensor("xk_t", [D, S], F32R, kind="ExternalInput")
    xv_t = nc.dram_tensor("xv_t", [D, S], F32R, kind="ExternalInput")
    wq_t = nc.dram_tensor("wq_t", [D, HL], F32R, kind="ExternalInput")
    wk_t = nc.dram_tensor("wk_t", [D, HL], F32R, kind="ExternalInput")
    wv_t = nc.dram_tensor("wv_t", [D, HL], F32R, kind="ExternalInput")
    wo_t = nc.dram_tensor("wo_t", [HL, D], F32R, kind="ExternalInput")
    bq = nc.dram_tensor("bq", [HL], F32, kind="ExternalInput")
    bk = nc.dram_tensor("bk", [HL], F32, kind="ExternalInput")
    masks = nc.dram_tensor("masks", [4, 128, 512], F32, kind="ExternalInput")
    partial_t = nc.dram_tensor("partial_t", [D, S], F32, kind="ExternalOutput")

    rs_dram = nc.dram_tensor("rs_scratch", [NHL, S], F32, kind="Internal")
    rr_dram = nc.dram_tensor("rr_scratch", [NHL, S], F32, kind="Internal")

    with tile.TileContext(nc) as tc:
        with tc.tile_pool(name="consts", bufs=1) as consts:
            mask_sb = consts.tile([128, 4, 512], F32, tag="masks")
            nc.sync.dma_start(out=mask_sb, in_=masks[:].rearrange("m p f -> p m f"))
            bq_sb = consts.tile([128, J_TILES], F32, tag="bq")
            bk_sb = consts.tile([128, J_TILES], F32, tag="bk")
            nc.sync.dma_start(out=bq_sb, in_=bq[:].rearrange("(t p) -> p t", p=128))
            nc.sync.dma_start(out=bk_sb, in_=bk[:].rearrange("(t p) -> p t", p=128))
            ones_sb = consts.tile([128, NHL], F32, tag="ones")
            nc.vector.memset(ones_sb, 1.0)

            with tc.tile_pool(name="otp", bufs=1) as otp:
                # O.T accumulates here (fp32r); normalized in place later.
                OT = [otp.tile([128, S], F32R, tag=f"ot{j}", name=f"ot{j}")
                      for j in range(J_TILES)]

                _projections_and_attention(
                    nc, tc, mask_sb, bq_sb, bk_sb, ones_sb, OT,
                    xq_t, xk_t, xv_t, wq_t, wk_t, wv_t, rs_dram, upto,
                )
                if upto == "all":
                    _normalize_and_outproj(
                        nc, tc, OT, rs_dram, rr_dram, wo_t, partial_t,
                    )

    nc.finalize()
    _nc_cache[key] = nc
    return nc


def _projections_and_attention(nc, tc, mask_sb, bq_sb, bk_sb, ones_sb, OT,
                               xq_t, xk_t, xv_t, wq_t, wk_t, wv_t, rs_dram,
                               upto="all"):
    with (
        tc.tile_pool(name="qk", bufs=1) as qkpool,
        tc.tile_pool(name="vpool", bufs=1) as vpool,
    ):
        QT = [qkpool.tile([128, S], F32R, tag=f"qt{j}", name=f"qt{j}")
              for j in range(J_TILES)]
        KT = [qkpool.tile([128, S], F32R, tag=f"kt{j}", name=f"kt{j}")
              for j in range(J_TILES)]
        V = [vpool.tile([128, NHL, HD + 1], F32R, tag=f"v{s}", name=f"v{s}")
             for s in range(S_TILES)]

        # ================= Q / K projections =================
        # QT[jt][:, sc] = Wq[d,jt].T @ x[d, sc]  accumulated over d-tiles
        for (w_dram, b_sb, out_tiles, x_dram, wtag) in (
            (wq_t, bq_sb, QT, xq_t, "wq"),
            (wk_t, bk_sb, KT, xk_t, "wk"),
        ):
            with (
                tc.tile_pool(name=wtag + "w", bufs=1) as wpool,
                tc.tile_pool(name=wtag + "x", bufs=1) as xpool,
                tc.tile_pool(name=wtag + "p", bufs=2, space="PSUM") as ppsum,
            ):
                # whole weight, rows contiguous: [128, dt, j]
                w_sb = wpool.tile([128, D_TILES, HL], F32R, tag=wtag, name=wtag)
                nc.sync.dma_start(
                    out=w_sb, in_=w_dram[:].rearrange("(dt p) j -> p dt j", p=128)
                )
                for sc in range(4):  # 512-wide s-chunks
                    scsl = slice(sc * 512, (sc + 1) * 512)
                    # two batched loads of 4 d-tiles each: [128, 4, 512]
                    xblks = []
                    for half in range(2):
                        xb = xpool.tile([128, 4, 512], F32R, tag=f"xb{half}",
                                        name=f"xb{half}", bufs=2)
                        nc.sync.dma_start(
                            out=xb,
                            in_=x_dram[half * 512:(half + 1) * 512, scsl]
                            .rearrange("(dt p) f -> p dt f", p=128),
                        )
                        xblks.append(xb)
                    for jt in range(J_TILES):
                        jsl = slice(jt * 128, (jt + 1) * 128)
                        ps = ppsum.tile([128, 512], F32, tag="pp")
                        for dt in range(D_TILES):
                            nc.tensor.matmul(
                                ps, w_sb[:, dt, jsl], xblks[dt // 4][:, dt % 4, :],
                                start=(dt == 0), stop=(dt == D_TILES - 1),
                            )
                        # psum + per-feature bias -> fp32r tile (DVE rounds)
                        nc.vector.tensor_scalar_add(
                            out_tiles[jt][:, scsl], ps, b_sb[:, jt:jt + 1]
                        )

        # ================= V projection =================
        with (
            tc.tile_pool(name="vw", bufs=1) as wpool,
            tc.tile_pool(name="vx", bufs=1) as xpool,
            tc.tile_pool(name="vpp", bufs=2, space="PSUM") as ppsum,
        ):
            wv_sb = wpool.tile([128, D_TILES, HL], F32R, tag="wv", name="wv")
            nc.sync.dma_start(
                out=wv_sb, in_=wv_t[:].rearrange("(dt p) j -> p dt j", p=128)
            )
            for st in range(S_TILES):
                ssl = slice(st * 128, (st + 1) * 128)
                # all 8 d-tiles of this s-block in one DMA: [128, 8, 128]
                xv_blk = xpool.tile([128, D_TILES, 128], F32R, tag="xvb",
                                    name="xvb", bufs=3)
                nc.sync.dma_start(
                    out=xv_blk,
                    in_=xv_t[:, ssl].rearrange("(dt p) f -> p dt f", p=128),
                )
                ps = ppsum.tile([128, HL], F32, tag="vp")
                for dt in range(D_TILES):
                    nc.tensor.matmul(
                        ps, xv_blk[:, dt, :], wv_sb[:, dt, :],
                        start=(dt == 0), stop=(dt == D_TILES - 1),
                    )
                nc.vector.tensor_copy(
                    V[st][:, :, 0:HD],
                    ps[:].rearrange("p (h x) -> p h x", h=NHL),
                )
                nc.vector.tensor_copy(V[st][:, :, HD], ones_sb)

        if upto == "proj":
            return

        # ================= attention =================
        # q processed in 1024-wide chunks (2 PSUM banks); k in 128-tiles.
        # Work is skipped at 512-column granularity outside the causal
        # triangle; the 4 mask patterns cover the 512-wide diagonal blocks.
        with (
            tc.tile_pool(name="scp", bufs=1, space="PSUM") as scp,
            tc.tile_pool(name="pvp", bufs=1, space="PSUM") as pvp,
            tc.tile_pool(name="ptp", bufs=2) as ptp,
            tc.tile_pool(name="osc", bufs=2) as osc,
        ):
            for hp in range(J_TILES):
                for Qi in range(QC):
                    q0 = Qi * 1024
                    qsl = slice(q0, q0 + 1024)
                    nk = 8 * (Qi + 1)
                    pv = [pvp.tile([128, 1024], F32, tag=f"pv{s}", name=f"pv{s}")
                          for s in range(2)]
                    for ki in range(nk):
                        k0 = ki * 128
                        ksl = slice(k0, k0 + 128)
                        # valid halves: k0 <= q0 + 512*half + 511
                        h_start = 0 if k0 <= q0 + 511 else 1
                        est = h_start * 512  # exp/matmul start column
                        st = [scp.tile([128, 1024], F32, tag=f"st{s}",
                                       name=f"st{s}") for s in range(2)]
                        pt = [ptp.tile([128, 1024], F32R, tag=f"pt{s}",
                                       name=f"pt{s}") for s in range(2)]
                        for s in range(2):  # head strip within the pair
                            psl = slice(s * 64, (s + 1) * 64)
                            tp = (64, 0) if s == 1 else None
                            for half in range(h_start, 2):
                                hsl = slice(half * 512, (half + 1) * 512)
                                qh = slice(q0 + half * 512, q0 + half * 512 + 512)
                                nc.tensor.matmul(
                                    st[s][:, hsl], KT[hp][psl, ksl],
                                    QT[hp][psl, qh],
                                    start=True, stop=True, tile_position=tp,
                                )
                                off = k0 - (q0 + half * 512)
                                if off >= 0:  # diagonal 512-block: causal mask
                                    nc.vector.tensor_add(
                                        st[s][:, hsl], st[s][:, hsl],
                                        mask_sb[:, off // 128, :],
                                    )
                            nc.scalar.activation(
                                pt[s][:, est:], st[s][:, est:], EXP, scale=0.125
                            )
                            h = 2 * hp + s
                            for half in range(h_start, 2):
                                hsl = slice(half * 512, (half + 1) * 512)
                                last = 8 * Qi + 3 if half == 0 else nk - 1
                                nc.tensor.matmul(
                                    pv[s][0:HD + 1, hsl], V[ki][:, h, :],
                                    pt[s][:, hsl],
                                    start=(ki == 0), stop=(ki == last),
                                )
                    for s in range(2):
                        sc = osc.tile([HD + 1, 1024], F32R, tag=f"os{s}",
                                      name=f"os{s}")
                        nc.vector.tensor_copy(sc, pv[s][0:HD + 1, :])
                        nc.sync.dma_start(out=OT[hp][s * 64:(s + 1) * 64, qsl],
                                          in_=sc[0:64, :])
                        nc.sync.dma_start(out=rs_dram[2 * hp + s, qsl],
                                          in_=sc[64:65, :].bitcast(F32))


def _normalize_and_outproj(nc, tc, OT, rs_dram, rr_dram, wo_t, partial_t):
    with tc.tile_pool(name="nrm", bufs=1) as nrm:
        rs_sb = nrm.tile([NHL, S], F32, tag="rs")
        nc.sync.dma_start(out=rs_sb, in_=rs_dram[:])
        rr_sb = nrm.tile([NHL, S], F32, tag="rr")
        nc.vector.reciprocal(rr_sb, rs_sb)
        nc.sync.dma_start(out=rr_dram[:], in_=rr_sb)
        for hp in range(J_TILES):
            rb = nrm.tile([128, S], F32, tag="rb", bufs=2)
            nc.sync.dma_start(
                out=rb[0:64, :],
                in_=rr_dram[2 * hp:2 * hp + 1, :].to_broadcast((64, S)),
            )
            nc.sync.dma_start(
                out=rb[64:128, :],
                in_=rr_dram[2 * hp + 1:2 * hp + 2, :].to_broadcast((64, S)),
            )
            # normalize O.T in place (fp32r read as fp32, re-rounded on write)
            nc.vector.tensor_mul(OT[hp], OT[hp].bitcast(F32), rb)

        # ================= output projection =================
        with (
            tc.tile_pool(name="wop", bufs=1) as wpool,
            tc.tile_pool(name="opp", bufs=2, space="PSUM") as ppsum,
        ):
            wo_sb = wpool.tile([128, J_TILES, D], F32R, tag="wo", name="wo")
            nc.sync.dma_start(
                out=wo_sb, in_=wo_t[:].rearrange("(jt p) d -> p jt d", p=128)
            )
            for dt in range(D_TILES):
                dsl = slice(dt * 128, (dt + 1) * 128)
                stage = nrm.tile([128, S], F32, tag="ostage", bufs=2)
                for sc in range(4):
                    scsl = slice(sc * 512, (sc + 1) * 512)
                    ps = ppsum.tile([128, 512], F32, tag="op")
                    for jt in range(J_TILES):
                        nc.tensor.matmul(
                            ps, wo_sb[:, jt, dsl], OT[jt][:, scsl],
                            start=(jt == 0), stop=(jt == J_TILES - 1),
                        )
                    nc.vector.tensor_copy(stage[:, scsl], ps)
                nc.sync.dma_start(out=partial_t[dsl, :], in_=stage)


def _round_f32r(a):
    """Round fp32 to fp32r (11 explicit mantissa bits, round-half-up)."""
    b = np.ascontiguousarray(a, np.float32).view(np.uint32)
    return ((b + np.uint32(0x800)) & np.uint32(0xFFFFF000)).view(np.float32)


def _make_masks():
    p = np.arange(128)[:, None]
    f = np.arange(512)[None, :]
    out = np.zeros((4, 128, 512), np.float32)
    for j in range(4):
        out[j] = np.where(128 * j + p > f, NEG, 0.0).astype(np.float32)
    return out


def kernel(q, k, v, padding_mask, W_q, b_q, W_k, b_k, W_v, b_v, W_o, b_o):
    q = np.asarray(q, np.float32)
    k = np.asarray(k, np.float32)
    v = np.asarray(v, np.float32)
    W_q = np.asarray(W_q, np.float32)
    W_k = np.asarray(W_k, np.float32)
    W_v = np.asarray(W_v, np.float32)
    W_o = np.asarray(W_o, np.float32)
    b_q = np.asarray(b_q, np.float32)
    b_k = np.asarray(b_k, np.float32)
    b_v = np.asarray(b_v, np.float32)
    b_o = np.asarray(b_o, np.float32)
    padding_mask = np.asarray(padding_mask)

    if padding_mask.any():
        return _numpy_reference(q, k, v, padding_mask, W_q, b_q, W_k, b_k,
                                W_v, b_v, W_o, b_o)

    nc = _build_nc()
    masks = _make_masks()
    in_maps = []
    for core in range(8):
        b, g = divmod(core, 2)
        gsl = slice(g * HL, (g + 1) * HL)
        in_maps.append({
            "xq_t": _round_f32r(q[b].T),
            "xk_t": _round_f32r(k[b].T),
            "xv_t": _round_f32r(v[b].T),
            "wq_t": _round_f32r(W_q[gsl].T),
            "wk_t": _round_f32r(W_k[gsl].T),
            "wv_t": _round_f32r(W_v[gsl].T),
            "wo_t": _round_f32r(W_o[:, gsl].T),
            "bq": np.ascontiguousarray(b_q[gsl]),
            "bk": np.ascontiguousarray(b_k[gsl]),
            "masks": masks,
        })

    res = run_bass_kernel_spmd(nc, in_maps, core_ids=list(range(8)))

    bias_vec = (W_o @ b_v + b_o).astype(np.float32)  # exact v/out bias folding
    out = np.empty((B, S, D), np.float32)
    for b in range(B):
        pt = res.results[2 * b]["partial_t"] + res.results[2 * b + 1]["partial_t"]
        out[b] = pt.T + bias_vec
    return out


def _numpy_reference(q, k, v, padding_mask, W_q, b_q, W_k, b_k, W_v, b_v,
                     W_o, b_o):
    """Slow exact path, only used when padding_mask is nonzero."""
    Q = (q @ W_q.T + b_q).reshape(B, S, NH, HD).transpose(0, 2, 1, 3)
    K = (k @ W_k.T + b_k).reshape(B, S, NH, HD).transpose(0, 2, 1, 3)
    Vv = (v @ W_v.T + b_v).reshape(B, S, NH, HD).transpose(0, 2, 1, 3)
    scores = np.einsum("bhqd,bhkd->bhqk", Q, K) / np.sqrt(HD)
    causal = np.triu(np.ones((S, S), bool), k=1)
    scores = np.where(causal, -np.inf, scores)
    scores = np.where(padding_mask[:, None, None, :], -np.inf, scores)
    scores = scores - scores.max(axis=-1, keepdims=True)
    e = np.exp(scores)
    attn = e / e.sum(axis=-1, keepdims=True)
    out = np.einsum("bhqk,bhkd->bhqd", attn, Vv)
    out = out.transpose(0, 2, 1, 3).reshape(B, S, H)
    return out @ W_o.T + b_o


# revision 7
# speedup vs baseline: 1.2948x; 1.2948x over previous
"""Multi-head attention kernel for 8 Trainium2 NeuronCores.

Problem: B=4, S=2048, D=H=1024, NH=16 heads (head_dim 64), causal MHA with
input projections (W_q/W_k/W_v), softmax, and output projection (W_o).

Sharding: 8 cores = 4 batches x 2 head-groups (tensor parallel over heads).
Each core computes, for one batch b and one group g of 8 heads:
  QT/KT = (x @ W{q,k}[g].T + b).T  stored [feature, seq]   (column-parallel)
  V     = x @ Wv[g].T              stored [seq, feature]
  per head: P.T = exp((K_h.T Q_h)/8 + causal_mask)  [k, q]
            O.T = V_aug_h.T @ P.T  (V augmented with a ones column -> row 64
                                    of the product is the softmax denominator)
  partial.T = (W_o[:, g] slice) contracted with normalized O.T (row-parallel)
The host sums the two group partials per batch, transposes, and adds
(W_o @ b_v + b_o)  (exact folding of the v/out biases).

All matmuls run in float32r (fp32 rounded to 11 explicit mantissa bits,
1 cycle/row on the PE when the moving dim >= 256 -> 4x faster than fp32,
~2.4e-4 relative rounding). Inputs are pre-rounded on the host so plain
HWDGE DMAs (no casting, no SWDGE descriptor-generation serialization) load
them.
"""

import sys

if "/opt/trn_rl_repo" not in sys.path:
    sys.path.insert(0, "/opt/trn_rl_repo")

import numpy as np

import concourse.mybir as mybir
import concourse.tile as tile
from concourse import bacc
from concourse.bass_utils import run_bass_kernel_spmd

F32 = mybir.dt.float32
F32R = mybir.dt.float32r
EXP = mybir.ActivationFunctionType.Exp

# Problem dims (full) and per-core dims
B, S, D, H, NH, HD = 4, 2048, 1024, 1024, 16, 64
HL = H // 2          # per-core feature width (8 heads x 64)
NHL = HL // HD       # 8 local heads
D_TILES = D // 128   # 8
J_TILES = HL // 128  # 4
S_TILES = S // 128   # 16
QC = S // 1024       # 2 big q-chunks (2 PSUM banks wide)
NEG = -1.0e30

_nc_cache = {}


def _build_nc(upto="all", reps=1):
    key = f"nc_{upto}_{reps}"
    if key in _nc_cache:
        return _nc_cache[key]

    nc = bacc.Bacc("TRN2", target_bir_lowering=False, debug=False)

    # fp32r inputs are pre-rounded fp32 bits from the host
    xq_t = nc.dram_tensor("xq_t", [D, S], F32R, kind="ExternalInput")
    xk_t = nc.dram_tensor("xk_t", [D, S], F32R, kind="ExternalInput")
    xv_t = nc.dram_tensor("xv_t", [D, S], F32R, kind="ExternalInput")
    wq_t = nc.dram_tensor("wq_t", [D, HL], F32R, kind="ExternalInput")
    wk_t = nc.dram_tensor("wk_t", [D, HL], F32R, kind="ExternalInput")
    wv_t = nc.dram_tensor("wv_t", [D, HL], F32R, kind="ExternalInput")
    wo_t = nc.dram_tensor("wo_t", [HL, D], F32R, kind="ExternalInput")
    bq = nc.dram_tensor("bq", [HL], F32, kind="ExternalInput")
    bk = nc.dram_tensor("bk", [HL], F32, kind="ExternalInput")
    masks = nc.dram_tensor("masks", [4, 128, 512], F32, kind="ExternalInput")
    partial_t = nc.dram_tensor("partial_t", [D, S], F32, kind="ExternalOutput")

    rs_dram = nc.dram_tensor("rs_scratch", [NHL, S], F32, kind="Internal")
    rr_dram = nc.dram_tensor("rr_scratch", [NHL, S], F32, kind="Internal")

    with tile.TileContext(nc) as tc:
        with tc.tile_pool(name="consts", bufs=1) as consts:
            mask_sb = consts.tile([128, 4, 512], F32, tag="masks")
            nc.sync.dma_start(out=mask_sb, in_=masks[:].rearrange("m p f -> p m f"))
            bq_sb = consts.tile([128, J_TILES], F32, tag="bq")
            bk_sb = consts.tile([128, J_TILES], F32, tag="bk")
            nc.sync.dma_start(out=bq_sb, in_=bq[:].rearrange("(t p) -> p t", p=128))
            nc.sync.dma_start(out=bk_sb, in_=bk[:].rearrange("(t p) -> p t", p=128))
            ones_sb = consts.tile([128, NHL], F32, tag="ones")
            nc.vector.memset(ones_sb, 1.0)

            with tc.tile_pool(name="otp", bufs=1) as otp:
                # O.T accumulates here (fp32r); normalized in place later.
                OT = [otp.tile([128, S], F32R, tag=f"ot{j}", name=f"ot{j}")
                      for j in range(J_TILES)]

                for _rep in range(reps):
                    _projections_and_attention(
                        nc, tc, mask_sb, bq_sb, bk_sb, ones_sb, OT,
                        xq_t, xk_t, xv_t, wq_t, wk_t, wv_t, rs_dram, upto,
                    )
                    if upto == "all":
                        _normalize_and_outproj(
                            nc, tc, OT, rs_dram, rr_dram, wo_t, partial_t,
                        )

    nc.finalize()
    _nc_cache[key] = nc
    return nc


def _projections_and_attention(nc, tc, mask_sb, bq_sb, bk_sb, ones_sb, OT,
                               xq_t, xk_t, xv_t, wq_t, wk_t, wv_t, rs_dram,
                               upto="all"):
    with (
        tc.tile_pool(name="qk", bufs=1) as qkpool,
        tc.tile_pool(name="vpool", bufs=1) as vpool,
    ):
        QT = [qkpool.tile([128, S], F32R, tag=f"qt{j}", name=f"qt{j}")
              for j in range(J_TILES)]
        KT = [qkpool.tile([128, S], F32R, tag=f"kt{j}", name=f"kt{j}")
              for j in range(J_TILES)]
        V = [vpool.tile([128, NHL, HD + 1], F32R, tag=f"v{s}", name=f"v{s}")
             for s in range(S_TILES)]

        # ================= Q / K projections =================
        # QT[jt][:, sc] = Wq[d,jt].T @ x[d, sc]  accumulated over d-tiles
        for (w_dram, b_sb, out_tiles, x_dram, wtag) in (
            (wq_t, bq_sb, QT, xq_t, "wq"),
            (wk_t, bk_sb, KT, xk_t, "wk"),
        ):
            with (
                tc.tile_pool(name=wtag + "w", bufs=1) as wpool,
                tc.tile_pool(name=wtag + "x", bufs=1) as xpool,
                tc.tile_pool(name=wtag + "p", bufs=2, space="PSUM") as ppsum,
            ):
                # whole weight, rows contiguous: [128, dt, j]
                w_sb = wpool.tile([128, D_TILES, HL], F32R, tag=wtag, name=wtag)
                nc.sync.dma_start(
                    out=w_sb, in_=w_dram[:].rearrange("(dt p) j -> p dt j", p=128)
                )
                for sc in range(4):  # 512-wide s-chunks
                    scsl = slice(sc * 512, (sc + 1) * 512)
                    # two batched loads of 4 d-tiles each: [128, 4, 512]
                    xblks = []
                    for half in range(2):
                        xb = xpool.tile([128, 4, 512], F32R, tag=f"xb{half}",
                                        name=f"xb{half}", bufs=2)
                        nc.sync.dma_start(
                            out=xb,
                            in_=x_dram[half * 512:(half + 1) * 512, scsl]
                            .rearrange("(dt p) f -> p dt f", p=128),
                        )
                        xblks.append(xb)
                    for jt in range(J_TILES):
                        jsl = slice(jt * 128, (jt + 1) * 128)
                        ps = ppsum.tile([128, 512], F32, tag="pp")
                        for dt in range(D_TILES):
                            nc.tensor.matmul(
                                ps, w_sb[:, dt, jsl], xblks[dt // 4][:, dt % 4, :],
                                start=(dt == 0), stop=(dt == D_TILES - 1),
                            )
                        # psum + per-feature bias -> fp32r tile (DVE rounds)
                        nc.vector.tensor_scalar_add(
                            out_tiles[jt][:, scsl], ps, b_sb[:, jt:jt + 1]
                        )

        # ================= V projection =================
        with (
            tc.tile_pool(name="vw", bufs=1) as wpool,
            tc.tile_pool(name="vx", bufs=1) as xpool,
            tc.tile_pool(name="vpp", bufs=2, space="PSUM") as ppsum,
        ):
            wv_sb = wpool.tile([128, D_TILES, HL], F32R, tag="wv", name="wv")
            nc.sync.dma_start(
                out=wv_sb, in_=wv_t[:].rearrange("(dt p) j -> p dt j", p=128)
            )
            for st in range(S_TILES):
                ssl = slice(st * 128, (st + 1) * 128)
                # all 8 d-tiles of this s-block in one DMA: [128, 8, 128]
                xv_blk = xpool.tile([128, D_TILES, 128], F32R, tag="xvb",
                                    name="xvb", bufs=3)
                nc.sync.dma_start(
                    out=xv_blk,
                    in_=xv_t[:, ssl].rearrange("(dt p) f -> p dt f", p=128),
                )
                ps = ppsum.tile([128, HL], F32, tag="vp")
                for dt in range(D_TILES):
                    nc.tensor.matmul(
                        ps, xv_blk[:, dt, :], wv_sb[:, dt, :],
                        start=(dt == 0), stop=(dt == D_TILES - 1),
                    )
                nc.vector.tensor_copy(
                    V[st][:, :, 0:HD],
                    ps[:].rearrange("p (h x) -> p h x", h=NHL),
                )
                nc.vector.tensor_copy(V[st][:, :, HD], ones_sb)

        if upto == "proj":
            return

        # ================= attention =================
        # q processed in 1024-wide chunks (2 PSUM banks); k in 128-tiles.
        # Work is skipped at 512-column granularity outside the causal
        # triangle; the 4 mask patterns cover the 512-wide diagonal blocks.
        with (
            tc.tile_pool(name="scp", bufs=1, space="PSUM") as scp,
            tc.tile_pool(name="pvp", bufs=1, space="PSUM") as pvp,
            tc.tile_pool(name="ptp", bufs=2) as ptp,
            tc.tile_pool(name="osc", bufs=2) as osc,
        ):
            for hp in range(J_TILES):
                for Qi in range(QC):
                    q0 = Qi * 1024
                    qsl = slice(q0, q0 + 1024)
                    nk = 8 * (Qi + 1)
                    pv = [pvp.tile([128, 1024], F32, tag=f"pv{s}", name=f"pv{s}")
                          for s in range(2)]
                    for ki in range(nk):
                        k0 = ki * 128
                        ksl = slice(k0, k0 + 128)
                        # valid halves: k0 <= q0 + 512*half + 511
                        h_start = 0 if k0 <= q0 + 511 else 1
                        est = h_start * 512  # exp/matmul start column
                        st = [scp.tile([128, 1024], F32, tag=f"st{s}",
                                       name=f"st{s}") for s in range(2)]
                        pt = [ptp.tile([128, 1024], F32R, tag=f"pt{s}",
                                       name=f"pt{s}") for s in range(2)]
                        for s in range(2):  # head strip within the pair
                            psl = slice(s * 64, (s + 1) * 64)
                            tp = (64, 0) if s == 1 else None
                            for half in range(h_start, 2):
                                hsl = slice(half * 512, (half + 1) * 512)
                                qh = slice(q0 + half * 512, q0 + half * 512 + 512)
                                nc.tensor.matmul(
                                    st[s][:, hsl], KT[hp][psl, ksl],
                                    QT[hp][psl, qh],
                                    start=True, stop=True, tile_position=tp,
                                )
                                off = k0 - (q0 + half * 512)
                                if off >= 0:  # diagonal 512-block: causal mask
                                    nc.vector.tensor_add(
                                        st[s][:, hsl], st[s][:, hsl],
                                        mask_sb[:, off // 128, :],
                                    )
                            nc.scalar.activation(
                                pt[s][:, est:], st[s][:, est:], EXP, scale=0.125
                            )
                            h = 2 * hp + s
                            for half in range(h_start, 2):
                                hsl = slice(half * 512, (half + 1) * 512)
                                last = 8 * Qi + 3 if half == 0 else nk - 1
                                nc.tensor.matmul(
                                    pv[s][0:HD + 1, hsl], V[ki][:, h, :],
                                    pt[s][:, hsl],
                                    start=(ki == 0), stop=(ki == last),
                                )
                    for s in range(2):
                        sc = osc.tile([HD + 1, 1024], F32R, tag=f"os{s}",
                                      name=f"os{s}")
                        nc.vector.tensor_copy(sc, pv[s][0:HD + 1, :])
                        nc.sync.dma_start(out=OT[hp][s * 64:(s + 1) * 64, qsl],
                                          in_=sc[0:64, :])
                        nc.sync.dma_start(out=rs_dram[2 * hp + s, qsl],
                                          in_=sc[64:65, :].bitcast(F32))


def _normalize_and_outproj(nc, tc, OT, rs_dram, rr_dram, wo_t, partial_t):
    with tc.tile_pool(name="nrm", bufs=1) as nrm:
        rs_sb = nrm.tile([NHL, S], F32, tag="rs")
        nc.sync.dma_start(out=rs_sb, in_=rs_dram[:])
        rr_sb = nrm.tile([NHL, S], F32, tag="rr")
        nc.vector.reciprocal(rr_sb, rs_sb)
        nc.sync.dma_start(out=rr_dram[:], in_=rr_sb)
        for hp in range(J_TILES):
            rb = nrm.tile([128, S], F32, tag="rb", bufs=2)
            nc.sync.dma_start(
                out=rb[0:64, :],
                in_=rr_dram[2 * hp:2 * hp + 1, :].to_broadcast((64, S)),
            )
            nc.sync.dma_start(
                out=rb[64:128, :],
                in_=rr_dram[2 * hp + 1:2 * hp + 2, :].to_broadcast((64, S)),
            )
            # normalize O.T in place (fp32r read as fp32, re-rounded on write)
            nc.vector.tensor_mul(OT[hp], OT[hp].bitcast(F32), rb)

        # ================= output projection =================
        with (
            tc.tile_pool(name="wop", bufs=1) as wpool,
            tc.tile_pool(name="opp", bufs=2, space="PSUM") as ppsum,
        ):
            wo_sb = wpool.tile([128, J_TILES, D], F32R, tag="wo", name="wo")
            nc.sync.dma_start(
                out=wo_sb, in_=wo_t[:].rearrange("(jt p) d -> p jt d", p=128)
            )
            for dt in range(D_TILES):
                dsl = slice(dt * 128, (dt + 1) * 128)
                stage = nrm.tile([128, S], F32, tag="ostage", bufs=2)
                for sc in range(4):
                    scsl = slice(sc * 512, (sc + 1) * 512)
                    ps = ppsum.tile([128, 512], F32, tag="op")
                    for jt in range(J_TILES):
                        nc.tensor.matmul(
                            ps, wo_sb[:, jt, dsl], OT[jt][:, scsl],
                            start=(jt == 0), stop=(jt == J_TILES - 1),
                        )
                    nc.vector.tensor_copy(stage[:, scsl], ps)
                nc.sync.dma_start(out=partial_t[dsl, :], in_=stage)


def _round_f32r(a):
    """Round fp32 to fp32r (11 explicit mantissa bits, round-half-up)."""
    b = np.ascontiguousarray(a, np.float32).view(np.uint32)
    return ((b + np.uint32(0x800)) & np.uint32(0xFFFFF000)).view(np.float32)


def _make_masks():
    p = np.arange(128)[:, None]
    f = np.arange(512)[None, :]
    out = np.zeros((4, 128, 512), np.float32)
    for j in range(4):
        out[j] = np.where(128 * j + p > f, NEG, 0.0).astype(np.float32)
    return out


def kernel(q, k, v, padding_mask, W_q, b_q, W_k, b_k, W_v, b_v, W_o, b_o):
    q = np.asarray(q, np.float32)
    k = np.asarray(k, np.float32)
    v = np.asarray(v, np.float32)
    W_q = np.asarray(W_q, np.float32)
    W_k = np.asarray(W_k, np.float32)
    W_v = np.asarray(W_v, np.float32)
    W_o = np.asarray(W_o, np.float32)
    b_q = np.asarray(b_q, np.float32)
    b_k = np.asarray(b_k, np.float32)
    b_v = np.asarray(b_v, np.float32)
    b_o = np.asarray(b_o, np.float32)
    padding_mask = np.asarray(padding_mask)

    if padding_mask.any():
        return _numpy_reference(q, k, v, padding_mask, W_q, b_q, W_k, b_k,
                                W_v, b_v, W_o, b_o)

    nc = _build_nc()
    masks = _make_masks()
    in_maps = []
    for core in range(8):
        b, g = divmod(core, 2)
        gsl = slice(g * HL, (g + 1) * HL)
        in_maps.append({
            "xq_t": _round_f32r(q[b].T),
            "xk_t": _round_f32r(k[b].T),
            "xv_t": _round_f32r(v[b].T),
            "wq_t": _round_f32r(W_q[gsl].T),
            "wk_t": _round_f32r(W_k[gsl].T),
            "wv_t": _round_f32r(W_v[gsl].T),
            "wo_t": _round_f32r(W_o[:, gsl].T),
            "bq": np.ascontiguousarray(b_q[gsl]),
            "bk": np.ascontiguousarray(b_k[gsl]),
            "masks": masks,
        })

    res = run_bass_kernel_spmd(nc, in_maps, core_ids=list(range(8)))

    bias_vec = (W_o @ b_v + b_o).astype(np.float32)  # exact v/out bias folding
    out = np.empty((B, S, D), np.float32)
    for b in range(B):
        pt = res.results[2 * b]["partial_t"] + res.results[2 * b + 1]["partial_t"]
        out[b] = pt.T + bias_vec
    return out


def _numpy_reference(q, k, v, padding_mask, W_q, b_q, W_k, b_k, W_v, b_v,
                     W_o, b_o):
    """Slow exact path, only used when padding_mask is nonzero."""
    Q = (q @ W_q.T + b_q).reshape(B, S, NH, HD).transpose(0, 2, 1, 3)
    K = (k @ W_k.T + b_k).reshape(B, S, NH, HD).transpose(0, 2, 1, 3)
    Vv = (v @ W_v.T + b_v).reshape(B, S, NH, HD).transpose(0, 2, 1, 3)
    scores = np.einsum("bhqd,bhkd->bhqk", Q, K) / np.sqrt(HD)
    causal = np.triu(np.ones((S, S), bool), k=1)
    scores = np.where(causal, -np.inf, scores)
    scores = np.where(padding_mask[:, None, None, :], -np.inf, scores)
    scores = scores - scores.max(axis=-1, keepdims=True)
    e = np.exp(scores)
    attn = e / e.sum(axis=-1, keepdims=True)
    out = np.einsum("bhqk,bhkd->bhqd", attn, Vv)
    out = out.transpose(0, 2, 1, 3).reshape(B, S, H)
    return out @ W_o.T + b_o


# revision 12
# speedup vs baseline: 5.7544x; 4.4443x over previous
"""Multi-head attention kernel for 8 Trainium2 NeuronCores.

Problem: B=4, S=2048, D=H=1024, NH=16 heads (head_dim 64), causal MHA with
input projections (W_q/W_k/W_v), softmax, and output projection (W_o).

Sharding: 8 cores = 4 batches x 2 head-groups (tensor parallel over heads).
Each core computes, for one batch b and one group g of 8 heads:
  QT/KT = (x @ W{q,k}[g].T + b).T  stored [feature, seq]   (column-parallel)
  V     = x @ Wv[g].T              stored [seq, feature]
  per head: P.T = exp((K_h.T Q_h)/8 + causal_mask)  [k, q]
            O.T = V_aug_h.T @ P.T  (V augmented with a ones column -> row 64
                                    of the product is the softmax denominator)
  partial.T = (W_o[:, g] slice) contracted with normalized O.T (row-parallel)
The host sums the two group partials per batch, transposes, and adds
(W_o @ b_v + b_o)  (exact folding of the v/out biases).

All matmuls run in float32r (fp32 rounded to 11 explicit mantissa bits,
1 cycle/row on the PE when the moving dim >= 256 -> 4x faster than fp32,
~2.4e-4 relative rounding). Inputs are pre-rounded on the host so plain
HWDGE DMAs (no casting, no SWDGE descriptor-generation serialization) load
them.
"""

import os
import sys

if "/opt/trn_rl_repo" not in sys.path:
    sys.path.insert(0, "/opt/trn_rl_repo")

ATTN_PROBE = os.environ.get("ATTN_PROBE", "")

import numpy as np

import concourse.mybir as mybir
import concourse.tile as tile
from concourse import bacc
from concourse.bass_utils import run_bass_kernel_spmd

F32 = mybir.dt.float32
F32R = mybir.dt.float32r
EXP = mybir.ActivationFunctionType.Exp

# Problem dims (full) and per-core dims
B, S, D, H, NH, HD = 4, 2048, 1024, 1024, 16, 64
HL = H // 2          # per-core feature width (8 heads x 64)
NHL = HL // HD       # 8 local heads
D_TILES = D // 128   # 8
J_TILES = HL // 128  # 4
S_TILES = S // 128   # 16
QC = S // 1024       # 2 big q-chunks (2 PSUM banks wide)
NEG = -1.0e30

_nc_cache = {}


def _build_nc(upto="all", reps=1):
    key = f"nc_{upto}_{reps}_{ATTN_PROBE}"
    if key in _nc_cache:
        return _nc_cache[key]

    nc = bacc.Bacc("TRN2", target_bir_lowering=False, debug=False)

    # fp32r inputs are pre-rounded fp32 bits from the host
    xq_t = nc.dram_tensor("xq_t", [D, S], F32R, kind="ExternalInput")
    xk_t = nc.dram_tensor("xk_t", [D, S], F32R, kind="ExternalInput")
    xv_t = nc.dram_tensor("xv_t", [D, S], F32R, kind="ExternalInput")
    wq_t = nc.dram_tensor("wq_t", [D, HL], F32R, kind="ExternalInput")
    wk_t = nc.dram_tensor("wk_t", [D, HL], F32R, kind="ExternalInput")
    wv_t = nc.dram_tensor("wv_t", [D, HL], F32R, kind="ExternalInput")
    wo_t = nc.dram_tensor("wo_t", [HL, D], F32R, kind="ExternalInput")
    bq = nc.dram_tensor("bq", [HL], F32, kind="ExternalInput")
    bk = nc.dram_tensor("bk", [HL], F32, kind="ExternalInput")
    masks = nc.dram_tensor("masks", [128, 1280], F32, kind="ExternalInput")
    partial_t = nc.dram_tensor("partial_t", [D, S], F32, kind="ExternalOutput")

    rs_dram = nc.dram_tensor("rs_scratch", [NHL, S], F32, kind="Internal")
    rr_dram = nc.dram_tensor("rr_scratch", [NHL, S], F32, kind="Internal")
    ot_dram = nc.dram_tensor("ot_scratch", [HL, S], F32R, kind="Internal")

    with tile.TileContext(nc) as tc:
        with tc.tile_pool(name="consts", bufs=1) as consts:
            mask_sb = consts.tile([128, 1280], F32, tag="masks")
            nc.sync.dma_start(out=mask_sb, in_=masks[:])
            bq_sb = consts.tile([128, J_TILES], F32, tag="bq")
            bk_sb = consts.tile([128, J_TILES], F32, tag="bk")
            nc.sync.dma_start(out=bq_sb, in_=bq[:].rearrange("(t p) -> p t", p=128))
            nc.sync.dma_start(out=bk_sb, in_=bk[:].rearrange("(t p) -> p t", p=128))
            ones_sb = consts.tile([128, NHL], F32, tag="ones")
            nc.vector.memset(ones_sb, 1.0)

            for _rep in range(reps):
                _projections_and_attention(
                    nc, tc, mask_sb, bq_sb, bk_sb, ones_sb, ot_dram,
                    xq_t, xk_t, xv_t, wq_t, wk_t, wv_t, rs_dram, upto,
                )
                if upto == "all":
                    _normalize_and_outproj(
                        nc, tc, ot_dram, rs_dram, rr_dram, wo_t, partial_t,
                    )

    nc.finalize()
    _nc_cache[key] = nc
    return nc


def _projections_and_attention(nc, tc, mask_sb, bq_sb, bk_sb, ones_sb, ot_dram,
                               xq_t, xk_t, xv_t, wq_t, wk_t, wv_t, rs_dram,
                               upto="all"):
    with (
        tc.tile_pool(name="qk", bufs=1) as qkpool,
        tc.tile_pool(name="vpool", bufs=1) as vpool,
    ):
        QT = [qkpool.tile([128, S], F32R, tag=f"qt{j}", name=f"qt{j}")
              for j in range(J_TILES)]
        # zero-padded K copies: KTz[j][0] has head-strip 0 rows (0:64) live
        # and rows 64:128 zero, KTz[j][1] the reverse -> K=128 score matmuls
        # contract the dead rows against zeros (full-array, no tile_position)
        KTz = [[qkpool.tile([128, S], F32R, tag=f"ktz{j}_{s}", name=f"ktz{j}_{s}")
                for s in range(2)] for j in range(J_TILES)]
        for j in range(J_TILES):
            nc.vector.memset(KTz[j][0][64:128, :].bitcast(F32), 0.0)
            nc.vector.memset(KTz[j][1][0:64, :].bitcast(F32), 0.0)
        V = [vpool.tile([128, NHL, HD + 1], F32R, tag=f"v{s}", name=f"v{s}")
             for s in range(S_TILES)]

        # ================= Q / K projections =================
        # QT[jt][:, sc] = Wq[d,jt].T @ x[d, sc]  accumulated over d-tiles
        for (w_dram, b_sb, out_tiles, x_dram, wtag) in (
            (wq_t, bq_sb, QT, xq_t, "wq"),
            (wk_t, bk_sb, KTz, xk_t, "wk"),
        ):
            with (
                tc.tile_pool(name=wtag + "w", bufs=1) as wpool,
                tc.tile_pool(name=wtag + "x", bufs=1) as xpool,
                tc.tile_pool(name=wtag + "p", bufs=2, space="PSUM") as ppsum,
            ):
                # whole weight, rows contiguous: [128, dt, j]
                w_sb = wpool.tile([128, D_TILES, HL], F32R, tag=wtag, name=wtag)
                nc.sync.dma_start(
                    out=w_sb, in_=w_dram[:].rearrange("(dt p) j -> p dt j", p=128)
                )
                for sc in range(4):  # 512-wide s-chunks
                    scsl = slice(sc * 512, (sc + 1) * 512)
                    # two batched loads of 4 d-tiles each: [128, 4, 512]
                    xblks = []
                    for half in range(2):
                        xb = xpool.tile([128, 4, 512], F32R, tag=f"xb{half}",
                                        name=f"xb{half}", bufs=2)
                        nc.sync.dma_start(
                            out=xb,
                            in_=x_dram[half * 512:(half + 1) * 512, scsl]
                            .rearrange("(dt p) f -> p dt f", p=128),
                        )
                        xblks.append(xb)
                    for jt in range(J_TILES):
                        jsl = slice(jt * 128, (jt + 1) * 128)
                        ps = ppsum.tile([128, 512], F32, tag="pp")
                        for dt in range(D_TILES):
                            nc.tensor.matmul(
                                ps, w_sb[:, dt, jsl], xblks[dt // 4][:, dt % 4, :],
                                start=(dt == 0), stop=(dt == D_TILES - 1),
                            )
                        # psum + per-feature bias -> fp32r tile (DVE rounds)
                        if wtag == "wk":
                            nc.vector.tensor_scalar_add(
                                out_tiles[jt][0][0:64, scsl], ps[0:64, :],
                                b_sb[0:64, jt:jt + 1],
                            )
                            nc.vector.tensor_scalar_add(
                                out_tiles[jt][1][64:128, scsl], ps[64:128, :],
                                b_sb[64:128, jt:jt + 1],
                            )
                        else:
                            nc.vector.tensor_scalar_add(
                                out_tiles[jt][:, scsl], ps, b_sb[:, jt:jt + 1]
                            )

        # ================= V projection =================
        with (
            tc.tile_pool(name="vw", bufs=1) as wpool,
            tc.tile_pool(name="vx", bufs=1) as xpool,
            tc.tile_pool(name="vpp", bufs=2, space="PSUM") as ppsum,
        ):
            wv_sb = wpool.tile([128, D_TILES, HL], F32R, tag="wv", name="wv")
            nc.sync.dma_start(
                out=wv_sb, in_=wv_t[:].rearrange("(dt p) j -> p dt j", p=128)
            )
            for st in range(S_TILES):
                ssl = slice(st * 128, (st + 1) * 128)
                # all 8 d-tiles of this s-block in one DMA: [128, 8, 128]
                xv_blk = xpool.tile([128, D_TILES, 128], F32R, tag="xvb",
                                    name="xvb", bufs=3)
                nc.sync.dma_start(
                    out=xv_blk,
                    in_=xv_t[:, ssl].rearrange("(dt p) f -> p dt f", p=128),
                )
                ps = ppsum.tile([128, HL], F32, tag="vp")
                for dt in range(D_TILES):
                    nc.tensor.matmul(
                        ps, xv_blk[:, dt, :], wv_sb[:, dt, :],
                        start=(dt == 0), stop=(dt == D_TILES - 1),
                    )
                nc.vector.tensor_copy(
                    V[st][:, :, 0:HD],
                    ps[:].rearrange("p (h x) -> p h x", h=NHL),
                )
                nc.vector.tensor_copy(V[st][:, :, HD], ones_sb)

        if upto == "proj":
            return

        # ================= attention =================
        # q processed in 1024-wide chunks (2-bank PV accumulators); k in
        # 128-tiles; work skipped at 512-column granularity outside the
        # causal triangle. Score matmuls use the zero-padded KTz copies so
        # both head strips run as full K=128 matmuls. PV matmuls trail the
        # score/exp pipeline by PIPE units so the PE never stalls on the
        # ACT engine's exp.
        PIPE = 4
        with (
            tc.tile_pool(name="scp", bufs=2, space="PSUM") as scp,
            tc.tile_pool(name="pvp", bufs=1, space="PSUM") as pvp,
            tc.tile_pool(name="ptp", bufs=3) as ptp,
            tc.tile_pool(name="osc", bufs=1) as osc,
        ):
            for hp in range(J_TILES):
                for Qi in range(QC):
                    q0 = Qi * 1024
                    qsl = slice(q0, q0 + 1024)
                    nk = 8 * (Qi + 1)
                    pv = [pvp.tile([128, 1024], F32, tag=f"pv{s}", name=f"pv{s}")
                          for s in range(2)]
                    pending = []

                    def flush_one():
                        pt_, s_, h_, hsl_, start_, stop_, ki_ = pending.pop(0)
                        nc.tensor.matmul(
                            pv[s_][0:HD + 1, hsl_], V[ki_][:, h_, :], pt_,
                            start=start_, stop=stop_,
                        )

                    for ki in range(nk):
                        k0 = ki * 128
                        ksl = slice(k0, k0 + 128)
                        h_start = 0 if k0 <= q0 + 511 else 1
                        for s in range(2):  # head strip within the pair
                            h = 2 * hp + s
                            for half in range(h_start, 2):
                                hsl = slice(half * 512, (half + 1) * 512)
                                qh = slice(q0 + half * 512, q0 + half * 512 + 512)
                                st = scp.tile([128, 512], F32, tag=f"st{s}",
                                              name=f"st{s}")
                                nc.tensor.matmul(
                                    st, KTz[hp][s][:, ksl], QT[hp][:, qh],
                                    start=True, stop=True,
                                )
                                off = k0 - (q0 + half * 512)
                                if off >= 0:  # diagonal 512-block: causal mask
                                    # mask pattern j is zero beyond (j+1)*128
                                    j = off // 128
                                    w = (j + 1) * 128
                                    nc.vector.tensor_add(
                                        st[:, 0:w], st[:, 0:w],
                                        mask_sb[:, MASK_OFF[j]:MASK_OFF[j] + w],
                                    )
                                pt = ptp.tile([128, 512], F32R, tag=f"pt{s}",
                                              name=f"pt{s}")
                                nc.scalar.activation(pt, st, EXP, scale=0.125)
                                last = 8 * Qi + 3 if half == 0 else nk - 1
                                pending.append(
                                    (pt, s, h, hsl, ki == 0, ki == last, ki))
                                if len(pending) > PIPE:
                                    flush_one()
                    while pending:
                        flush_one()
                    for s in range(2):
                        sc = osc.tile([HD + 1, 1024], F32R, tag=f"os{s}",
                                      name=f"os{s}")
                        nc.vector.tensor_copy(sc, pv[s][0:HD + 1, :])
                        nc.sync.dma_start(
                            out=ot_dram[hp * 128 + s * 64:hp * 128 + s * 64 + 64,
                                        qsl],
                            in_=sc[0:64, :])
                        nc.sync.dma_start(out=rs_dram[2 * hp + s, qsl],
                                          in_=sc[64:65, :].bitcast(F32))


def _normalize_and_outproj(nc, tc, ot_dram, rs_dram, rr_dram, wo_t, partial_t):
    with tc.tile_pool(name="nrm", bufs=1) as nrm:
        rs_sb = nrm.tile([NHL, S], F32, tag="rs")
        nc.sync.dma_start(out=rs_sb, in_=rs_dram[:])
        rr_sb = nrm.tile([NHL, S], F32, tag="rr")
        nc.vector.reciprocal(rr_sb, rs_sb)
        nc.sync.dma_start(out=rr_dram[:], in_=rr_sb)
        OT = []
        for hp in range(J_TILES):
            otn = nrm.tile([128, S], F32R, tag=f"otn{hp}", name=f"otn{hp}")
            nc.sync.dma_start(out=otn,
                              in_=ot_dram[hp * 128:(hp + 1) * 128, :])
            rb = nrm.tile([128, S], F32, tag="rb", bufs=2)
            nc.sync.dma_start(
                out=rb[0:64, :],
                in_=rr_dram[2 * hp:2 * hp + 1, :].to_broadcast((64, S)),
            )
            nc.sync.dma_start(
                out=rb[64:128, :],
                in_=rr_dram[2 * hp + 1:2 * hp + 2, :].to_broadcast((64, S)),
            )
            # normalize O.T in place (fp32r read as fp32, re-rounded on write)
            nc.vector.tensor_mul(otn, otn.bitcast(F32), rb)
            OT.append(otn)

        # ================= output projection =================
        with (
            tc.tile_pool(name="wop", bufs=1) as wpool,
            tc.tile_pool(name="opp", bufs=2, space="PSUM") as ppsum,
        ):
            wo_sb = wpool.tile([128, J_TILES, D], F32R, tag="wo", name="wo")
            nc.sync.dma_start(
                out=wo_sb, in_=wo_t[:].rearrange("(jt p) d -> p jt d", p=128)
            )
            for dt in range(D_TILES):
                dsl = slice(dt * 128, (dt + 1) * 128)
                stage = nrm.tile([128, S], F32, tag="ostage", bufs=2)
                for sc in range(4):
                    scsl = slice(sc * 512, (sc + 1) * 512)
                    ps = ppsum.tile([128, 512], F32, tag="op")
                    for jt in range(J_TILES):
                        nc.tensor.matmul(
                            ps, wo_sb[:, jt, dsl], OT[jt][:, scsl],
                            start=(jt == 0), stop=(jt == J_TILES - 1),
                        )
                    nc.vector.tensor_copy(stage[:, scsl], ps)
                nc.sync.dma_start(out=partial_t[dsl, :], in_=stage)


def _round_f32r(a):
    """Round fp32 to fp32r (11 explicit mantissa bits, round-half-up)."""
    b = np.ascontiguousarray(a, np.float32).view(np.uint32)
    return ((b + np.uint32(0x800)) & np.uint32(0xFFFFF000)).view(np.float32)


MASK_OFF = [0, 128, 384, 768]  # packed offsets; pattern j has width (j+1)*128


def _make_masks():
    p = np.arange(128)[:, None]
    out = np.zeros((128, 1280), np.float32)
    for j in range(4):
        w = (j + 1) * 128
        f = np.arange(w)[None, :]
        out[:, MASK_OFF[j]:MASK_OFF[j] + w] = np.where(
            128 * j + p > f, NEG, 0.0)
    return out


def kernel(q, k, v, padding_mask, W_q, b_q, W_k, b_k, W_v, b_v, W_o, b_o):
    q = np.asarray(q, np.float32)
    k = np.asarray(k, np.float32)
    v = np.asarray(v, np.float32)
    W_q = np.asarray(W_q, np.float32)
    W_k = np.asarray(W_k, np.float32)
    W_v = np.asarray(W_v, np.float32)
    W_o = np.asarray(W_o, np.float32)
    b_q = np.asarray(b_q, np.float32)
    b_k = np.asarray(b_k, np.float32)
    b_v = np.asarray(b_v, np.float32)
    b_o = np.asarray(b_o, np.float32)
    padding_mask = np.asarray(padding_mask)

    if padding_mask.any():
        return _numpy_reference(q, k, v, padding_mask, W_q, b_q, W_k, b_k,
                                W_v, b_v, W_o, b_o)

    nc = _build_nc()
    masks = _make_masks()
    in_maps = []
    for core in range(8):
        b, g = divmod(core, 2)
        gsl = slice(g * HL, (g + 1) * HL)
        in_maps.append({
            "xq_t": _round_f32r(q[b].T),
            "xk_t": _round_f32r(k[b].T),
            "xv_t": _round_f32r(v[b].T),
            "wq_t": _round_f32r(W_q[gsl].T),
            "wk_t": _round_f32r(W_k[gsl].T),
            "wv_t": _round_f32r(W_v[gsl].T),
            "wo_t": _round_f32r(W_o[:, gsl].T),
            "bq": np.ascontiguousarray(b_q[gsl]),
            "bk": np.ascontiguousarray(b_k[gsl]),
            "masks": masks,
        })

    res = run_bass_kernel_spmd(nc, in_maps, core_ids=list(range(8)))

    bias_vec = (W_o @ b_v + b_o).astype(np.float32)  # exact v/out bias folding
    out = np.empty((B, S, D), np.float32)
    for b in range(B):
        pt = res.results[2 * b]["partial_t"] + res.results[2 * b + 1]["partial_t"]
        out[b] = pt.T + bias_vec
    return out


def _numpy_reference(q, k, v, padding_mask, W_q, b_q, W_k, b_k, W_v, b_v,
                     W_o, b_o):
    """Slow exact path, only used when padding_mask is nonzero."""
    Q = (q @ W_q.T + b_q).reshape(B, S, NH, HD).transpose(0, 2, 1, 3)
    K = (k @ W_k.T + b_k).reshape(B, S, NH, HD).transpose(0, 2, 1, 3)
    Vv = (v @ W_v.T + b_v).reshape(B, S, NH, HD).transpose(0, 2, 1, 3)
    scores = np.einsum("bhqd,bhkd->bhqk", Q, K) / np.sqrt(HD)
    causal = np.triu(np.ones((S, S), bool), k=1)
    scores = np.where(causal, -np.inf, scores)
    scores = np.where(padding_mask[:, None, None, :], -np.inf, scores)
    scores = scores - scores.max(axis=-1, keepdims=True)
    e = np.exp(scores)
    attn = e / e.sum(axis=-1, keepdims=True)
    out = np.einsum("bhqk,bhkd->bhqd", attn, Vv)
    out = out.transpose(0, 2, 1, 3).reshape(B, S, H)
    return out @ W_o.T + b_o
